# revision 23
# baseline (speedup 1.0000x reference)
"""Trainium2 Bass kernel for the CapsuleLayer dynamic-routing module.

Hybrid sharding (8 NeuronCores), v2:
  - Iteration 1 is FULLY REPLICATED: c1 is uniform (softmax of zeros), so
    s1[b,nu] = (1/J) sum_{(i,j)} W[(i,j),nu] x[b,(i,j)] is a pure linear
    map computed on every core with 144 full-width matmuls (k=128, m=128).
    No collective needed; this overlaps the NEFF entry barrier + input DMA.
  - Iterations 2..3 are J-SHARDED: each core owns jl = 144 of the 1152
    in_channels, rows packed r = jl*8 + i -> exactly 9 chunks of 128
    partitions. Per iteration:
      a-pass:  C[r,nu] = sum_b x[b,r] v[b,nu]  (18 matmuls, k=128);
               z = W.*C; fold u on DVE; fold i via one block-ones matmul
               (S16[p,g] = [p//8==g]) -> a[jl,n] local to the core.
      c-pass:  b += a (local); expb = exp(b) [16,90]; D_loc via ones-matmul;
               cexp replicated to the (jl,i) partition pattern via R16
               matmul; A = W_loc .* cexp  (all local, tiny).
      s-pass:  s_partial[b,nu] = sum_r A[r,nu] x[b,r] (18 matmuls).
      ONE AllReduce carries [s_partial | D_loc] ([128, 330] f32); the
      softmax denominator is folded in AFTER the collective (squash input
      s = s_sum * (1/D[n])), so no second collective is needed.
  - Only 2 AllReduces total and every matmul runs with full 128-wide
    partition utilization; the agreement-pass DVE volume is 8x smaller
    than the batch-parallel layout.

Host pre-layouts (not measured):
  wf  [128, 72*160]  bf16  W[(j,i) rows, (n,u)]  r = t*128+p = j*8+i
  xt  [128, 72*256]  bf16  x^T[(j,i) rows, b]
  xtl [128, 9*256]   bf16  per-core slice of xt (chunks 9c..9c+9)
  wl  [128, 9*160]   bf16  per-core slice of wf
  xb  [128, 2*9*128] bf16  x[b=bh*128+p, r_loc = t*128+q]
  s16 [128, 16] bf16; r16 [16, 128] bf16 (block-ones i-fold constants)
"""

import numpy as np

B, I, J, N, U = 256, 8, 1152, 10, 16
NU = N * U            # 160
ITERS = 3
NCORES = 8
JL = J // NCORES      # 144 local j per core
TG = 72               # global (j,i) row chunks of 128
TL = 9                # local row chunks of 128
GD = 16               # jl groups per chunk (128/8)

_CACHE = {}


def _build_nc():
    import concourse.bass as bass
    import concourse.bacc as bacc
    import concourse.tile as tile
    from concourse import mybir

    f32 = mybir.dt.float32
    bf16 = mybir.dt.bfloat16
    AL = mybir.AluOpType
    AF = mybir.ActivationFunctionType
    AX = mybir.AxisListType

    nc = bacc.Bacc("TRN2", target_bir_lowering=False, debug=False,
                   num_devices=NCORES)
    wf_d = nc.dram_tensor("wf", [128, TG * NU], bf16, kind="ExternalInput").ap()
    xt_d = nc.dram_tensor("xt", [128, TG * B], bf16, kind="ExternalInput").ap()
    xb_d = nc.dram_tensor("xb", [128, 2 * TL * 128], bf16,
                          kind="ExternalInput").ap()
    s16_d = nc.dram_tensor("s16", [128, GD], bf16, kind="ExternalInput").ap()
    r16_d = nc.dram_tensor("r16", [GD, 128], bf16, kind="ExternalInput").ap()
    v_d = nc.dram_tensor("v", [GD, 2 * NU], f32, kind="ExternalOutput").ap()

    with tile.TileContext(nc) as tc:
        with (
            tc.tile_pool(name="big", bufs=1) as big,
            tc.tile_pool(name="abp", bufs=1) as abp,
            tc.tile_pool(name="small", bufs=2) as small,
            tc.tile_pool(name="pers", bufs=1) as pers,
            tc.tile_pool(name="ps_s", bufs=1, space="PSUM") as ps_s,
            tc.tile_pool(name="ps_c", bufs=2, space="PSUM") as ps_c,
            tc.tile_pool(name="ps_m", bufs=1, space="PSUM") as ps_m,
            tc.tile_pool(name="ps_w", bufs=1, space="PSUM") as ps_w,
            tc.tile_pool(name="dram", bufs=4, space="DRAM") as dram,
        ):
            # ---------------- streamed input loads ----------------
            # s1-pass consumes (W, xT) group g as soon as its DMA lands.
            wf_v = wf_d.rearrange("p (t c) -> p t c", t=TG)
            xt_v = xt_d.rearrange("p (t b) -> p t b", t=TG)
            WF = big.tile([128, TG, NU], bf16)
            XGT = []
            nc.sync.dma_start(out=WF, in_=wf_v)
            for g in range(3):
                xgt = big.tile([128, 24, B], bf16, tag=f"xg{g}")
                nc.sync.dma_start(out=xgt, in_=xt_v[:, 24 * g:24 * g + 24])
                XGT.append(xgt)
            XB = big.tile([128, 2, TL, 128], bf16)
            nc.sync.dma_start(out=XB, in_=xb_d.rearrange(
                "p (h t q) -> p h t q", h=2, t=TL))
            S16 = pers.tile([128, GD], bf16)
            nc.sync.dma_start(out=S16, in_=s16_d)
            R16 = pers.tile([GD, 128], bf16)
            nc.sync.dma_start(out=R16, in_=r16_d)
            # local-slice views: per-core chunk order puts the core's own
            # 9 (j,i)-row chunks first in wf/xt.
            WL = WF[:, 0:TL].rearrange("p t (n u) -> p t n u", n=N)
            XTL = XGT[0][:, 0:TL]

            ones16 = pers.tile([GD, 128], bf16)
            nc.vector.memset(ones16, 1.0)
            bmat = pers.tile([GD, TL * N], f32)       # b[g,(t,n)], jl=t*16+g
            nc.vector.memset(bmat, 0.0)

            # PE warm-up fodder (keeps the HAM clock up through DMA and
            # collective waits).
            warm_rhs = pers.tile([GD, NU], bf16)
            nc.vector.memset(warm_rhs, 0.0)

            def warm_pe(count):
                pw = ps_w.tile([128, NU], f32, tag="warm")
                for _ in range(count):
                    nc.tensor.matmul(pw, lhsT=ones16, rhs=warm_rhs,
                                     start=True, stop=True)

            warm_pe(20)

            # ---------------- squash helper ----------------
            def squash(s_sc, last):
                """s_sc: [128, 2, N, U] f32 (already 1/D-scaled).
                Returns vb16 [128,2,NU] bf16 (with 1/B folded) or DMAs v."""
                sq = small.tile([128, 2, N, U], f32, tag="sq")
                nc.vector.tensor_tensor(out=sq, in0=s_sc, in1=s_sc,
                                        op=AL.mult)
                mag = small.tile([128, 2, U], f32, tag="mag")
                nc.vector.tensor_reduce(
                    out=mag, in_=sq.rearrange("p h n u -> p h u n"),
                    axis=AX.X, op=AL.add)
                sqrtm = small.tile([128, 2, U], f32, tag="sqrtm")
                nc.scalar.activation(out=sqrtm, in_=mag, func=AF.Sqrt)
                onep = small.tile([128, 2, U], f32, tag="onep")
                nc.vector.tensor_scalar_add(out=onep, in0=mag, scalar1=1.0)
                rec = small.tile([128, 2, U], f32, tag="rec")
                nc.vector.reciprocal(out=rec, in_=onep)
                gg = small.tile([128, 2, U], f32, tag="gg")
                if last:
                    nc.vector.tensor_tensor(out=gg, in0=sqrtm, in1=rec,
                                            op=AL.mult)
                    vf = small.tile([128, 2, N, U], f32, tag="vf")
                    nc.vector.tensor_tensor(
                        out=vf, in0=s_sc,
                        in1=gg.unsqueeze(2).broadcast_to([128, 2, N, U]),
                        op=AL.mult)
                    nc.sync.dma_start(
                        out=v_d, in_=vf.rearrange("p h n u -> p (h n u)"))
                    return None
                nc.vector.scalar_tensor_tensor(
                    out=gg, in0=sqrtm, scalar=1.0 / B, in1=rec,
                    op0=AL.mult, op1=AL.mult)
                vb16 = small.tile([128, 2, N, U], bf16, tag="vb16")
                nc.vector.tensor_tensor(
                    out=vb16, in0=s_sc,
                    in1=gg.unsqueeze(2).broadcast_to([128, 2, N, U]),
                    op=AL.mult)
                return vb16.rearrange("p h n u -> p h (n u)")

            # last-iteration squash on the ReduceScatter slice [16, ...]
            def squash16(s_sc):
                sq = small.tile([GD, 2, N, U], f32, tag="sq6")
                nc.vector.tensor_tensor(out=sq, in0=s_sc, in1=s_sc,
                                        op=AL.mult)
                mag = small.tile([GD, 2, U], f32, tag="mag6")
                nc.vector.tensor_reduce(
                    out=mag, in_=sq.rearrange("p h n u -> p h u n"),
                    axis=AX.X, op=AL.add)
                sqrtm = small.tile([GD, 2, U], f32, tag="sqm6")
                nc.scalar.activation(out=sqrtm, in_=mag, func=AF.Sqrt)
                onep = small.tile([GD, 2, U], f32, tag="onp6")
                nc.vector.tensor_scalar_add(out=onep, in0=mag, scalar1=1.0)
                rec = small.tile([GD, 2, U], f32, tag="rec6")
                nc.vector.reciprocal(out=rec, in_=onep)
                gg = small.tile([GD, 2, U], f32, tag="gg6")
                nc.vector.tensor_tensor(out=gg, in0=sqrtm, in1=rec,
                                        op=AL.mult)
                vf = small.tile([GD, 2, N, U], f32, tag="vf6")
                nc.vector.tensor_tensor(
                    out=vf, in0=s_sc,
                    in1=gg.unsqueeze(2).broadcast_to([GD, 2, N, U]),
                    op=AL.mult)
                nc.sync.dma_start(
                    out=v_d, in_=vf.rearrange("p h n u -> p (h n u)"))

            # ------- fused agreement + routing + s-pass (pipelined) -------
            # Per 3-chunk group: C matmuls -> z=W.*C -> u-fold -> i-fold
            # (S16 matmul) -> b+=a slice -> exp -> replicate (R16 matmul)
            # -> A=W.*cexp -> next-iteration s matmuls. Groups pipeline
            # across engines; D accumulates via a 3-part ones-matmul.
            def fused_pass(vb16):
                """vb16 [128, 2, NU] (1/B folded). Returns (pss, pay)."""
                pss = ps_s.tile([128, 2, NU], f32)
                pay = small.tile([128, 2 * NU + N], bf16, tag="pay")
                psD = ps_m.tile([128, 3 * N], f32, tag="psD")
                A3s = []
                for t3 in range(3):
                    psc = ps_c.tile([128, 3, NU], f32)
                    for g in range(3):
                        t = t3 * 3 + g
                        nc.tensor.matmul(psc[:, g], lhsT=XB[:, 0, t],
                                         rhs=vb16[:, 0],
                                         start=True, stop=False)
                        nc.tensor.matmul(psc[:, g], lhsT=XB[:, 1, t],
                                         rhs=vb16[:, 1],
                                         start=False, stop=True)
                    z3 = small.tile([128, 3, N, U], bf16, tag="z3")
                    nc.vector.tensor_tensor(
                        out=z3, in0=WL[:, t3 * 3:t3 * 3 + 3],
                        in1=psc.rearrange("p t c -> p t (c)")
                        .rearrange("p t (n u) -> p t n u", n=N),
                        op=AL.mult)
                    zu3 = small.tile([128, 3, N], bf16, tag="zu3")
                    with nc.allow_low_precision(
                            reason="16-term u-fold; bf16 agreement wire "
                                   "precision is within tolerance"):
                        nc.vector.tensor_reduce(
                            out=zu3, in_=z3, axis=AX.X, op=AL.add)
                    psa3 = ps_m.tile([GD, 3 * N], f32, tag="psa3")
                    nc.tensor.matmul(psa3, lhsT=S16,
                                     rhs=zu3.rearrange("p t n -> p (t n)"),
                                     start=True, stop=True)
                    bsl = bmat[:, 3 * N * t3:3 * N * (t3 + 1)]
                    nc.vector.tensor_tensor(out=bsl, in0=bsl, in1=psa3,
                                            op=AL.add)
                    expb3 = small.tile([GD, 3 * N], bf16, tag="expb3")
                    nc.scalar.activation(out=expb3, in_=bsl, func=AF.Exp)
                    eb3 = small.tile([GD, N], bf16, tag="eb3")
                    with nc.allow_low_precision(
                            reason="3-term exp-sum; bf16 wire for D is "
                                   "within tolerance"):
                        nc.vector.tensor_reduce(
                            out=eb3,
                            in_=expb3.rearrange("g (t n) -> g n t", t=3),
                            axis=AX.X, op=AL.add)
                    nc.tensor.matmul(psD[:, N * t3:N * (t3 + 1)],
                                     lhsT=ones16, rhs=eb3,
                                     start=True, stop=True)
                    psC3 = ps_m.tile([128, 3 * N], f32, tag="psC3")
                    nc.tensor.matmul(psC3, lhsT=R16, rhs=expb3,
                                     start=True, stop=True)
                    cexpb3 = small.tile([128, 3 * N], bf16, tag="cexpb3")
                    nc.scalar.copy(out=cexpb3, in_=psC3)
                    A3 = abp.tile([128, 3, N, U], bf16, tag=f"A3{t3}")
                    nc.vector.tensor_tensor(
                        out=A3, in0=WL[:, t3 * 3:t3 * 3 + 3],
                        in1=cexpb3.rearrange("p (t n) -> p t n", t=3)
                        .unsqueeze(3).broadcast_to([128, 3, N, U]),
                        op=AL.mult)
                    A3s.append(A3)
                # contiguous s-pass accumulation (groups must not nest)
                for bh in range(2):
                    for t in range(TL):
                        nc.tensor.matmul(
                            pss[:, bh],
                            lhsT=XTL[:, t, 128 * bh:128 * bh + 128],
                            rhs=A3s[t // 3][:, t % 3]
                            .rearrange("p n u -> p (n u)"),
                            start=(t == 0), stop=(t == TL - 1))
                psDs = small.tile([128, 3 * N], f32, tag="psDs")
                nc.scalar.copy(out=psDs, in_=psD)
                with nc.allow_low_precision(
                        reason="3-term D-sum; bf16 wire is within tolerance"):
                    nc.vector.tensor_reduce(
                        out=pay[:, 2 * NU:],
                        in_=psDs.rearrange("p (t n) -> p n t", t=3),
                        axis=AX.X, op=AL.add)
                nc.scalar.copy(out=pay[:, 0:2 * NU],
                               in_=pss.rearrange("p h c -> p (h c)"))
                return pss, pay

            # ---------------- iteration 1: replicated ----------------
            pss1 = ps_s.tile([128, 2, NU], f32)
            for g in range(3):
                for ch in range(24):
                    t = 24 * g + ch
                    for bh in range(2):
                        nc.tensor.matmul(
                            pss1[:, bh],
                            lhsT=XGT[g][:, ch, 128 * bh:128 * bh + 128],
                            rhs=WF[:, t],
                            start=(t == 0), stop=(t == TG - 1))
            s_sc1 = small.tile([128, 2, N, U], f32, tag="ssc")
            nc.scalar.activation(
                out=s_sc1, in_=pss1.rearrange("p h c -> p h (c)")
                .rearrange("p h (n u) -> p h n u", n=N),
                func=AF.Copy, scale=1.0 / J)
            vb1 = squash(s_sc1, last=False)
            pss2, pay2 = fused_pass(vb1)

            # ---------------- AllReduce (s2 + D2) ----------------
            ar_in2 = dram.tile([128, 2 * NU + N], bf16, tag="arin2")
            ar_out2 = dram.tile([128, 2 * NU + N], bf16, tag="arout2")
            nc.sync.dma_start(out=ar_in2, in_=pay2)
            nc.gpsimd.collective_compute(
                "AllReduce", AL.add,
                ins=[ar_in2.opt()], outs=[ar_out2.opt()],
                replica_groups=[list(range(NCORES))])
            warm_pe(12)
            rbuf = small.tile([128, 2 * NU + N], bf16, tag="rbuf")
            nc.sync.dma_start(out=rbuf, in_=ar_out2)
            dinv = small.tile([128, N], f32, tag="dinv")
            nc.vector.reciprocal(out=dinv, in_=rbuf[:, 2 * NU:])
            s_sc2 = small.tile([128, 2, N, U], f32, tag="ssc")
            nc.vector.tensor_tensor(
                out=s_sc2,
                in0=rbuf[:, 0:2 * NU].rearrange(
                    "p (h n u) -> p h n u", h=2, n=N),
                in1=dinv.unsqueeze(1).unsqueeze(3)
                .broadcast_to([128, 2, N, U]),
                op=AL.mult)
            vb2 = squash(s_sc2, last=False)
            pss3, pay3 = fused_pass(vb2)

            # ---------------- ReduceScatter (s3 + D3) ----------------
            ar_in3 = dram.tile([128, 2 * NU + N], bf16, tag="arin3")
            rs_out = dram.tile([GD, 2 * NU + N], bf16, tag="rsout")
            nc.sync.dma_start(out=ar_in3, in_=pay3)
            nc.gpsimd.collective_compute(
                "ReduceScatter", AL.add,
                ins=[ar_in3.opt()], outs=[rs_out.opt()],
                replica_groups=[list(range(NCORES))])
            rbuf6 = small.tile([GD, 2 * NU + N], bf16, tag="rsbuf")
            nc.sync.dma_start(out=rbuf6, in_=rs_out)
            dinv6 = small.tile([GD, N], f32, tag="dinvl")
            nc.vector.reciprocal(out=dinv6, in_=rbuf6[:, 2 * NU:])
            s_sc6 = small.tile([GD, 2, N, U], f32, tag="sscl")
            nc.vector.tensor_tensor(
                out=s_sc6,
                in0=rbuf6[:, 0:2 * NU].rearrange(
                    "p (h n u) -> p h n u", h=2, n=N),
                in1=dinv6.unsqueeze(1).unsqueeze(3)
                .broadcast_to([GD, 2, N, U]),
                op=AL.mult)
            squash16(s_sc6)

    nc.compile()
    return nc


def _prep_inputs(x_full, W):
    """Host-side relayout. x_full: [B, I, J] f32, W: [J, N, U, I] f32."""
    import ml_dtypes
    bf = ml_dtypes.bfloat16

    # global rows r = j*8 + i
    # wf[p, t, (n,u)] = W[j, n, u, i],  r = t*128+p
    wf = np.ascontiguousarray(
        W.transpose(0, 3, 1, 2).reshape(J * I, NU)      # [r, nu]
        .reshape(TG, 128, NU).transpose(1, 0, 2)
    ).reshape(128, TG * NU).astype(bf)
    # xt[p, t, b] = x[b, i, j]
    xr = np.ascontiguousarray(
        x_full.transpose(2, 1, 0).reshape(J * I, B))    # [r, b]
    xt = np.ascontiguousarray(
        xr.reshape(TG, 128, B).transpose(1, 0, 2)
    ).reshape(128, TG * B).astype(bf)

    s16 = (np.arange(128)[:, None] // 8 ==
           np.arange(GD)[None, :]).astype(bf)
    r16 = np.ascontiguousarray(s16.T)

    wf3 = wf.reshape(128, TG, NU)
    xt3 = xt.reshape(128, TG, B)
    in_maps = []
    for c in range(NCORES):
        # per-core chunk order: own 9 chunks first (commutative accumulation)
        order = list(range(9 * c, 9 * c + 9)) + \
            [t for t in range(TG) if not (9 * c <= t < 9 * c + 9)]
        wf_c = np.ascontiguousarray(wf3[:, order]).reshape(128, TG * NU)
        xt_c = np.ascontiguousarray(xt3[:, order]).reshape(128, TG * B)
        # xb[p, bh, t, q] = x[b=bh*128+p, i, j=144c+jl], r_loc = t*128+q
        xc = x_full[:, :, JL * c:JL * c + JL]            # [B, I, JL]
        xb = np.ascontiguousarray(
            xc.transpose(0, 2, 1).reshape(B, JL * I)     # [b, r_loc]
            .reshape(2, 128, TL, 128).transpose(1, 0, 2, 3)
        ).reshape(128, 2 * TL * 128).astype(bf)
        in_maps.append({"wf": wf_c, "xt": xt_c,
                        "xb": xb, "s16": s16, "r16": r16})
    return in_maps


def _assemble(results):
    """Core c holds partition rows [16c, 16c+16) of the [128, 2, NU] v."""
    o = np.concatenate(
        [np.asarray(r["v"], dtype=np.float32).reshape(GD, 2, NU)
         for r in results], axis=0)                      # [128, 2, NU]
    v = np.ascontiguousarray(o.transpose(1, 0, 2)).reshape(B, N, U, 1)
    return v


def kernel(x, W):
    """x: [256, 8, 1152] f32; W: [1152, 10, 16, 8] f32 ->
    v: [256, 10, 16, 1] f32."""
    from concourse.bass_utils import run_bass_kernel_spmd

    x = np.asarray(x, dtype=np.float32)
    W = np.asarray(W, dtype=np.float32)
    if "nc" not in _CACHE:
        _CACHE["nc"] = _build_nc()
    nc = _CACHE["nc"]
    in_maps = _prep_inputs(x, W)
    res = run_bass_kernel_spmd(nc, in_maps, core_ids=list(range(NCORES)))
    return _assemble(res.results)


if __name__ == "__main__":
    rng = np.random.default_rng(0)
    x = rng.standard_normal((B, I, J), dtype=np.float32)
    W = rng.standard_normal((J, N, U, I), dtype=np.float32)
    got = kernel(x, W)
    u_hat = np.einsum('jnui,bij->bjnu', W, x)
    b = np.zeros((J, N), dtype=np.float32)
    for _ in range(ITERS):
        e = np.exp(b - b.max(axis=0, keepdims=True))
        c = e / e.sum(axis=0, keepdims=True)
        s = np.einsum('jn,bjnu->bnu', c, u_hat)
        mag = np.sum(s * s, axis=1, keepdims=True)
        v = (mag / (1.0 + mag)) * (s / np.sqrt(mag))
        b = b + np.einsum('bjnu,bnu->jn', u_hat, v) / B
    exp = v[..., None]
    rel = np.linalg.norm(got - exp) / np.linalg.norm(exp)
    print("rel_fro:", rel)


# revision 24
# speedup vs baseline: 1.2004x; 1.2004x over previous
"""Trainium2 Bass kernel for the CapsuleLayer dynamic-routing module.

Hybrid sharding (8 NeuronCores), v2:
  - Iteration 1 is FULLY REPLICATED: c1 is uniform (softmax of zeros), so
    s1[b,nu] = (1/J) sum_{(i,j)} W[(i,j),nu] x[b,(i,j)] is a pure linear
    map computed on every core with 144 full-width matmuls (k=128, m=128).
    No collective needed; this overlaps the NEFF entry barrier + input DMA.
  - Iterations 2..3 are J-SHARDED: each core owns jl = 144 of the 1152
    in_channels, rows packed r = jl*8 + i -> exactly 9 chunks of 128
    partitions. Per iteration:
      a-pass:  C[r,nu] = sum_b x[b,r] v[b,nu]  (18 matmuls, k=128);
               z = W.*C; fold u on DVE; fold i via one block-ones matmul
               (S16[p,g] = [p//8==g]) -> a[jl,n] local to the core.
      c-pass:  b += a (local); expb = exp(b) [16,90]; D_loc via ones-matmul;
               cexp replicated to the (jl,i) partition pattern via R16
               matmul; A = W_loc .* cexp  (all local, tiny).
      s-pass:  s_partial[b,nu] = sum_r A[r,nu] x[b,r] (18 matmuls).
      ONE AllReduce carries [s_partial | D_loc] ([128, 330] f32); the
      softmax denominator is folded in AFTER the collective (squash input
      s = s_sum * (1/D[n])), so no second collective is needed.
  - Only 2 AllReduces total and every matmul runs with full 128-wide
    partition utilization; the agreement-pass DVE volume is 8x smaller
    than the batch-parallel layout.

Host pre-layouts (not measured):
  wf  [128, 72*160]  bf16  W[(j,i) rows, (n,u)]  r = t*128+p = j*8+i
  xt  [128, 72*256]  bf16  x^T[(j,i) rows, b]
  xtl [128, 9*256]   bf16  per-core slice of xt (chunks 9c..9c+9)
  wl  [128, 9*160]   bf16  per-core slice of wf
  xb  [128, 2*9*128] bf16  x[b=bh*128+p, r_loc = t*128+q]
  s16 [128, 16] bf16; r16 [16, 128] bf16 (block-ones i-fold constants)
"""

import numpy as np

B, I, J, N, U = 256, 8, 1152, 10, 16
NU = N * U            # 160
ITERS = 3
NCORES = 8
JL = J // NCORES      # 144 local j per core
TG = 72               # global (j,i) row chunks of 128
TL = 9                # local row chunks of 128
GD = 16               # jl groups per chunk (128/8)

_CACHE = {}


def _build_nc():
    import concourse.bass as bass
    import concourse.bacc as bacc
    import concourse.tile as tile
    from concourse import mybir

    f32 = mybir.dt.float32
    bf16 = mybir.dt.bfloat16
    AL = mybir.AluOpType
    AF = mybir.ActivationFunctionType
    AX = mybir.AxisListType

    nc = bacc.Bacc("TRN2", target_bir_lowering=False, debug=False,
                   num_devices=NCORES)
    wf_d = nc.dram_tensor("wf", [128, TG * NU], bf16, kind="ExternalInput").ap()
    xt_d = nc.dram_tensor("xt", [128, TG * B], bf16, kind="ExternalInput").ap()
    xb_d = nc.dram_tensor("xb", [128, 2 * TL * 128], bf16,
                          kind="ExternalInput").ap()
    s16_d = nc.dram_tensor("s16", [128, GD], bf16, kind="ExternalInput").ap()
    r16_d = nc.dram_tensor("r16", [GD, 128], bf16, kind="ExternalInput").ap()
    v_d = nc.dram_tensor("v", [GD, 2 * NU], f32, kind="ExternalOutput").ap()

    with tile.TileContext(nc) as tc:
        with (
            tc.tile_pool(name="big", bufs=1) as big,
            tc.tile_pool(name="abp", bufs=1) as abp,
            tc.tile_pool(name="small", bufs=2) as small,
            tc.tile_pool(name="pers", bufs=1) as pers,
            tc.tile_pool(name="ps_s", bufs=1, space="PSUM") as ps_s,
            tc.tile_pool(name="ps_c", bufs=3, space="PSUM") as ps_c,
            tc.tile_pool(name="ps_m", bufs=1, space="PSUM") as ps_m,
            tc.tile_pool(name="ps_w", bufs=1, space="PSUM") as ps_w,
            tc.tile_pool(name="dram", bufs=4, space="DRAM") as dram,
        ):
            # ---------------- streamed input loads ----------------
            # s1-pass consumes (W, xT) group g as soon as its DMA lands.
            wf_v = wf_d.rearrange("p (t c) -> p t c", t=TG)
            xt_v = xt_d.rearrange("p (t b) -> p t b", t=TG)
            WF = big.tile([128, TG, NU], bf16)
            XGT = []
            nc.sync.dma_start(out=WF, in_=wf_v)
            for g in range(3):
                xgt = big.tile([128, 24, B], bf16, tag=f"xg{g}")
                nc.sync.dma_start(out=xgt, in_=xt_v[:, 24 * g:24 * g + 24])
                XGT.append(xgt)
            XB = big.tile([128, 2, TL, 128], bf16)
            nc.sync.dma_start(out=XB, in_=xb_d.rearrange(
                "p (h t q) -> p h t q", h=2, t=TL))
            S16 = pers.tile([128, GD], bf16)
            nc.sync.dma_start(out=S16, in_=s16_d)
            R16 = pers.tile([GD, 128], bf16)
            nc.sync.dma_start(out=R16, in_=r16_d)
            # local-slice views: per-core chunk order puts the core's own
            # 9 (j,i)-row chunks first in wf/xt.
            WL = WF[:, 0:TL].rearrange("p t (n u) -> p t n u", n=N)
            XTL = XGT[0][:, 0:TL]

            ones16 = pers.tile([GD, 128], bf16)
            nc.vector.memset(ones16, 1.0)
            bmat = pers.tile([GD, TL * N], f32)       # b[g,(t,n)], jl=t*16+g

            # PE warm-up fodder (keeps the HAM clock up through DMA and
            # collective waits).
            warm_rhs = pers.tile([GD, NU], bf16)
            nc.vector.memset(warm_rhs, 0.0)

            def warm_pe(count):
                pw = ps_w.tile([128, NU], f32, tag="warm")
                for _ in range(count):
                    nc.tensor.matmul(pw, lhsT=ones16, rhs=warm_rhs,
                                     start=True, stop=True)

            warm_pe(20)

            # ---------------- squash helper ----------------
            def squash(s_sc, last):
                """s_sc: [128, 2, N, U] f32 (already 1/D-scaled).
                Returns vb16 [128,2,NU] bf16 (with 1/B folded) or DMAs v."""
                sq = small.tile([128, 2, N, U], f32, tag="sq")
                nc.vector.tensor_tensor(out=sq, in0=s_sc, in1=s_sc,
                                        op=AL.mult)
                mag = small.tile([128, 2, U], f32, tag="mag")
                nc.vector.tensor_reduce(
                    out=mag, in_=sq.rearrange("p h n u -> p h u n"),
                    axis=AX.X, op=AL.add)
                sqrtm = small.tile([128, 2, U], f32, tag="sqrtm")
                nc.scalar.activation(out=sqrtm, in_=mag, func=AF.Sqrt)
                onep = small.tile([128, 2, U], f32, tag="onep")
                nc.vector.tensor_scalar_add(out=onep, in0=mag, scalar1=1.0)
                rec = small.tile([128, 2, U], f32, tag="rec")
                nc.vector.reciprocal(out=rec, in_=onep)
                gg = small.tile([128, 2, U], f32, tag="gg")
                if last:
                    nc.vector.tensor_tensor(out=gg, in0=sqrtm, in1=rec,
                                            op=AL.mult)
                    vf = small.tile([128, 2, N, U], f32, tag="vf")
                    nc.vector.tensor_tensor(
                        out=vf, in0=s_sc,
                        in1=gg.unsqueeze(2).broadcast_to([128, 2, N, U]),
                        op=AL.mult)
                    nc.sync.dma_start(
                        out=v_d, in_=vf.rearrange("p h n u -> p (h n u)"))
                    return None
                nc.vector.scalar_tensor_tensor(
                    out=gg, in0=sqrtm, scalar=1.0 / B, in1=rec,
                    op0=AL.mult, op1=AL.mult)
                vb16 = small.tile([128, 2, N, U], bf16, tag="vb16")
                nc.vector.tensor_tensor(
                    out=vb16, in0=s_sc,
                    in1=gg.unsqueeze(2).broadcast_to([128, 2, N, U]),
                    op=AL.mult)
                return vb16.rearrange("p h n u -> p h (n u)")

            # last-iteration squash on the ReduceScatter slice [16, ...]
            def squash16(s_sc):
                sq = small.tile([GD, 2, N, U], f32, tag="sq6")
                nc.vector.tensor_tensor(out=sq, in0=s_sc, in1=s_sc,
                                        op=AL.mult)
                mag = small.tile([GD, 2, U], f32, tag="mag6")
                nc.vector.tensor_reduce(
                    out=mag, in_=sq.rearrange("p h n u -> p h u n"),
                    axis=AX.X, op=AL.add)
                sqrtm = small.tile([GD, 2, U], f32, tag="sqm6")
                nc.scalar.activation(out=sqrtm, in_=mag, func=AF.Sqrt)
                onep = small.tile([GD, 2, U], f32, tag="onp6")
                nc.vector.tensor_scalar_add(out=onep, in0=mag, scalar1=1.0)
                rec = small.tile([GD, 2, U], f32, tag="rec6")
                nc.vector.reciprocal(out=rec, in_=onep)
                gg = small.tile([GD, 2, U], f32, tag="gg6")
                nc.vector.tensor_tensor(out=gg, in0=sqrtm, in1=rec,
                                        op=AL.mult)
                vf = small.tile([GD, 2, N, U], f32, tag="vf6")
                nc.vector.tensor_tensor(
                    out=vf, in0=s_sc,
                    in1=gg.unsqueeze(2).broadcast_to([GD, 2, N, U]),
                    op=AL.mult)
                nc.sync.dma_start(
                    out=v_d, in_=vf.rearrange("p h n u -> p (h n u)"))

            # ---------------- a-pass helper (j-sharded) ----------------
            def a_pass(vb16):
                """vb16 [128, 2, NU] (1/B folded). Returns psum a [16, 90]."""
                zu = small.tile([128, TL, N], bf16, tag="zu")
                for t3 in range(3):
                    psc = ps_c.tile([128, 3, NU], f32)
                    for g in range(3):
                        t = t3 * 3 + g
                        nc.tensor.matmul(psc[:, g], lhsT=XB[:, 0, t],
                                         rhs=vb16[:, 0],
                                         start=True, stop=False)
                        nc.tensor.matmul(psc[:, g], lhsT=XB[:, 1, t],
                                         rhs=vb16[:, 1],
                                         start=False, stop=True)
                    z3 = small.tile([128, 3, N, U], bf16, tag="z3")
                    nc.vector.tensor_tensor(
                        out=z3, in0=WL[:, t3 * 3:t3 * 3 + 3],
                        in1=psc.rearrange("p t c -> p t (c)")
                        .rearrange("p t (n u) -> p t n u", n=N),
                        op=AL.mult)
                    with nc.allow_low_precision(
                            reason="16-term u-fold; bf16 agreement wire "
                                   "precision is within tolerance"):
                        nc.vector.tensor_reduce(
                            out=zu[:, t3 * 3:t3 * 3 + 3], in_=z3,
                            axis=AX.X, op=AL.add)
                psa = ps_m.tile([GD, TL * N], f32, tag="psa")
                nc.tensor.matmul(psa, lhsT=S16,
                                 rhs=zu.rearrange("p t n -> p (t n)"),
                                 start=True, stop=True)
                return psa

            # ---------------- c-pass helper: b -> (A, psD) ----------------
            def c_pass():
                expb = small.tile([GD, TL * N], bf16, tag="expb")
                nc.scalar.activation(out=expb, in_=bmat, func=AF.Exp)
                eb2b = small.tile([GD, N], bf16, tag="eb2b")
                with nc.allow_low_precision(
                        reason="9-term exp-sum; bf16 wire for D is within "
                               "tolerance"):
                    nc.vector.tensor_reduce(
                        out=eb2b,
                        in_=expb.rearrange("g (t n) -> g n t", t=TL),
                        axis=AX.X, op=AL.add)
                psDC = ps_m.tile([128, N + TL * N], f32, tag="psDC")
                psD = psDC[:, 0:N]
                nc.tensor.matmul(psD, lhsT=ones16, rhs=eb2b,
                                 start=True, stop=True)
                psC = psDC[:, N:]
                nc.tensor.matmul(psC, lhsT=R16, rhs=expb,
                                 start=True, stop=True)
                cexpb = small.tile([128, TL * N], bf16, tag="cexpb")
                nc.scalar.copy(out=cexpb, in_=psC)
                A = abp.tile([128, TL, N, U], bf16, tag="A")
                nc.vector.tensor_tensor(
                    out=A, in0=WL,
                    in1=cexpb.rearrange("p (t n) -> p t n", t=TL)
                    .unsqueeze(3).broadcast_to([128, TL, N, U]),
                    op=AL.mult)
                return A, psD

            # ---------------- iteration 1: replicated ----------------
            pss1 = ps_s.tile([128, 2, NU], f32)
            for g in range(3):
                for ch in range(24):
                    t = 24 * g + ch
                    for bh in range(2):
                        nc.tensor.matmul(
                            pss1[:, bh],
                            lhsT=XGT[g][:, ch, 128 * bh:128 * bh + 128],
                            rhs=WF[:, t],
                            start=(t == 0), stop=(t == TG - 1))
            s_sc1 = small.tile([128, 2, N, U], f32, tag="ssc")
            nc.scalar.activation(
                out=s_sc1, in_=pss1.rearrange("p h c -> p h (c)")
                .rearrange("p h (n u) -> p h n u", n=N),
                func=AF.Copy, scale=1.0 / J)
            vb1 = squash(s_sc1, last=False)
            psa1 = a_pass(vb1)
            nc.vector.tensor_copy(out=bmat, in_=psa1)

            # ---------------- iterations 2..3: j-sharded ----------------
            for it in range(1, ITERS):
                last = it == ITERS - 1
                A, psD = c_pass()
                pay = small.tile([128, 2 * NU + N], bf16, tag="pay")
                nc.scalar.copy(out=pay[:, 2 * NU:], in_=psD)
                pss = ps_s.tile([128, 2, NU], f32)
                for bh in range(2):
                    for t in range(TL):
                        nc.tensor.matmul(
                            pss[:, bh],
                            lhsT=XTL[:, t, 128 * bh:128 * bh + 128],
                            rhs=A[:, t].rearrange("p n u -> p (n u)"),
                            start=(t == 0), stop=(t == TL - 1))
                nc.scalar.copy(out=pay[:, 0:2 * NU],
                               in_=pss.rearrange("p h c -> p (h c)"))
                ar_in = dram.tile([128, 2 * NU + N], bf16, tag=f"arin{it}")
                ar_out = None
                if not last:
                    ar_out = dram.tile([128, 2 * NU + N], bf16,
                                       tag=f"arout{it}")
                nc.sync.dma_start(out=ar_in, in_=pay)
                if last:
                    rs_out = dram.tile([GD, 2 * NU + N], bf16, tag="rsout")
                    nc.gpsimd.collective_compute(
                        "ReduceScatter", AL.add,
                        ins=[ar_in.opt()], outs=[rs_out.opt()],
                        replica_groups=[list(range(NCORES))])
                    warm_pe(12)
                    rbuf = small.tile([GD, 2 * NU + N], bf16, tag="rsbuf")
                    nc.sync.dma_start(out=rbuf, in_=rs_out)
                    dinv = small.tile([GD, N], f32, tag="dinvl")
                    nc.vector.reciprocal(out=dinv, in_=rbuf[:, 2 * NU:])
                    s_sc = small.tile([GD, 2, N, U], f32, tag="sscl")
                    nc.vector.tensor_tensor(
                        out=s_sc,
                        in0=rbuf[:, 0:2 * NU].rearrange(
                            "p (h n u) -> p h n u", h=2, n=N),
                        in1=dinv.unsqueeze(1).unsqueeze(3)
                        .broadcast_to([GD, 2, N, U]),
                        op=AL.mult)
                    squash16(s_sc)
                    break
                nc.gpsimd.collective_compute(
                    "AllReduce", AL.add,
                    ins=[ar_in.opt()], outs=[ar_out.opt()],
                    replica_groups=[list(range(NCORES))])
                warm_pe(12)
                rbuf = small.tile([128, 2 * NU + N], bf16, tag="rbuf")
                nc.sync.dma_start(out=rbuf, in_=ar_out)
                dinv = small.tile([128, N], f32, tag="dinv")
                nc.vector.reciprocal(out=dinv, in_=rbuf[:, 2 * NU:])
                s_sc = small.tile([128, 2, N, U], f32, tag="ssc")
                nc.vector.tensor_tensor(
                    out=s_sc,
                    in0=rbuf[:, 0:2 * NU].rearrange(
                        "p (h n u) -> p h n u", h=2, n=N),
                    in1=dinv.unsqueeze(1).unsqueeze(3)
                    .broadcast_to([128, 2, N, U]),
                    op=AL.mult)
                vb = squash(s_sc, last=False)
                psa = a_pass(vb)
                nc.vector.tensor_tensor(out=bmat, in0=bmat, in1=psa,
                                        op=AL.add)

    nc.compile()
    return nc


def _prep_inputs(x_full, W):
    """Host-side relayout. x_full: [B, I, J] f32, W: [J, N, U, I] f32."""
    import ml_dtypes
    bf = ml_dtypes.bfloat16

    # global rows r = j*8 + i
    # wf[p, t, (n,u)] = W[j, n, u, i],  r = t*128+p
    wf = np.ascontiguousarray(
        W.transpose(0, 3, 1, 2).reshape(J * I, NU)      # [r, nu]
        .reshape(TG, 128, NU).transpose(1, 0, 2)
    ).reshape(128, TG * NU).astype(bf)
    # xt[p, t, b] = x[b, i, j]
    xr = np.ascontiguousarray(
        x_full.transpose(2, 1, 0).reshape(J * I, B))    # [r, b]
    xt = np.ascontiguousarray(
        xr.reshape(TG, 128, B).transpose(1, 0, 2)
    ).reshape(128, TG * B).astype(bf)

    s16 = (np.arange(128)[:, None] // 8 ==
           np.arange(GD)[None, :]).astype(bf)
    r16 = np.ascontiguousarray(s16.T)

    wf3 = wf.reshape(128, TG, NU)
    xt3 = xt.reshape(128, TG, B)
    in_maps = []
    for c in range(NCORES):
        # per-core chunk order: own 9 chunks first (commutative accumulation)
        order = list(range(9 * c, 9 * c + 9)) + \
            [t for t in range(TG) if not (9 * c <= t < 9 * c + 9)]
        wf_c = np.ascontiguousarray(wf3[:, order]).reshape(128, TG * NU)
        xt_c = np.ascontiguousarray(xt3[:, order]).reshape(128, TG * B)
        # xb[p, bh, t, q] = x[b=bh*128+p, i, j=144c+jl], r_loc = t*128+q
        xc = x_full[:, :, JL * c:JL * c + JL]            # [B, I, JL]
        xb = np.ascontiguousarray(
            xc.transpose(0, 2, 1).reshape(B, JL * I)     # [b, r_loc]
            .reshape(2, 128, TL, 128).transpose(1, 0, 2, 3)
        ).reshape(128, 2 * TL * 128).astype(bf)
        in_maps.append({"wf": wf_c, "xt": xt_c,
                        "xb": xb, "s16": s16, "r16": r16})
    return in_maps


def _assemble(results):
    """Core c holds partition rows [16c, 16c+16) of the [128, 2, NU] v."""
    o = np.concatenate(
        [np.asarray(r["v"], dtype=np.float32).reshape(GD, 2, NU)
         for r in results], axis=0)                      # [128, 2, NU]
    v = np.ascontiguousarray(o.transpose(1, 0, 2)).reshape(B, N, U, 1)
    return v


def kernel(x, W):
    """x: [256, 8, 1152] f32; W: [1152, 10, 16, 8] f32 ->
    v: [256, 10, 16, 1] f32."""
    from concourse.bass_utils import run_bass_kernel_spmd

    x = np.asarray(x, dtype=np.float32)
    W = np.asarray(W, dtype=np.float32)
    if "nc" not in _CACHE:
        _CACHE["nc"] = _build_nc()
    nc = _CACHE["nc"]
    in_maps = _prep_inputs(x, W)
    res = run_bass_kernel_spmd(nc, in_maps, core_ids=list(range(NCORES)))
    return _assemble(res.results)


if __name__ == "__main__":
    rng = np.random.default_rng(0)
    x = rng.standard_normal((B, I, J), dtype=np.float32)
    W = rng.standard_normal((J, N, U, I), dtype=np.float32)
    got = kernel(x, W)
    u_hat = np.einsum('jnui,bij->bjnu', W, x)
    b = np.zeros((J, N), dtype=np.float32)
    for _ in range(ITERS):
        e = np.exp(b - b.max(axis=0, keepdims=True))
        c = e / e.sum(axis=0, keepdims=True)
        s = np.einsum('jn,bjnu->bnu', c, u_hat)
        mag = np.sum(s * s, axis=1, keepdims=True)
        v = (mag / (1.0 + mag)) * (s / np.sqrt(mag))
        b = b + np.einsum('bjnu,bnu->jn', u_hat, v) / B
    exp = v[..., None]
    rel = np.linalg.norm(got - exp) / np.linalg.norm(exp)
    print("rel_fro:", rel)
